# revision 57
# baseline (speedup 1.0000x reference)
"""Trainium2 Bass kernel for nn_AttentionBlock (GroupNorm + spatial
self-attention + residual), data-parallel over batch across 8 NeuronCores.

Self-contained: patches the container's concourse runtime (walrus here only
accepts 1 sync wait per instruction; LDWEIGHTS dedupe; optional NTFF
profiling), builds the Tile kernel, shards inputs 4 samples/core, runs SPMD
on cores 0-7, gathers the full output.

Math per sample (x: [C=256, N=1024]):
  h  = GN_8groups(x) * gamma + beta                    [C, N]
  u  = (16 wq^T wk)^T h            (qk fused: s = q.k = h^T (wk^T wq) h)
  M  = ((16 wo wv) h)^T                                [N, C]  (out-proj fused)
  sT[j,i] = sum_b h[b,j] u[b,i]                        (j on partitions)
  Pu = exp(sT/256)         (scores are tiny; no max subtraction needed)
  ou[o,i] = sum_j M[j,o] Pu[j,i]
  r_bc[p,i] = sum_j Pu[j,i]  (ones-matmul with 128 ones columns ->
                              the row-sum lands pre-broadcast on all
                              128 partitions; no separate bcast matmul)
  rinv = exp(-ln r - ln 16)            (fast ACT tables; folds the 1/16)
  out = x + ou * rinv_bc

GroupNorm group sums use one block-diagonal [128,128] selector matmul that
returns the per-group sums already broadcast per-channel. All large matmuls
run in bf16/fp8 (fp32 accumulate in PSUM); GroupNorm statistics and the
final combine are fp32. Emission is software-pipelined across the 4 samples
so GroupNorm/projection chains hide under attention matmuls.
"""
import contextlib
import ctypes
import math
import os
import sys
import types

sys.path.insert(0, '/opt/trn_rl_repo')

import numpy as np

import bass_rust
import concourse.bass as bass
import concourse.tile as tile
from concourse import mybir

F32 = mybir.dt.float32
F32R = mybir.dt.float32r
BF16 = mybir.dt.bfloat16
FP8 = mybir.dt.float8e4
DR = mybir.MatmulPerfMode.DoubleRow
AX = mybir.AxisListType.X
AF = mybir.ActivationFunctionType
OP = mybir.AluOpType

C = 256
N = 1024
G = 8
EPS = 1e-5
WSCALE = 16.0          # fp8-range scaling folded into wu / wov
SCALE_FUSED = 1.0 / (16.0 * WSCALE)   # exp scale, fused-qk build
SCALE_PLAIN = 1.0 / 16.0              # exp scale, separate-q/k build
NB = 4   # samples per core
NCORES = 8
NC2 = C // 128
NJ = N // 128
NIC = N // 512

_installed = [False]
_split_counter = [0]
_last_exec_time_ns = [None]


def _make_ntff_hook(so_path):
    lib = ctypes.CDLL(so_path)
    lib.axon_start_nrt_profile.argtypes = [ctypes.POINTER(ctypes.c_int64), ctypes.c_size_t]
    lib.axon_start_nrt_profile.restype = ctypes.c_int64
    lib.axon_stop_nrt_profile.argtypes = [ctypes.c_char_p]
    lib.axon_stop_nrt_profile.restype = ctypes.c_int64

    @contextlib.contextmanager
    def _hook(output_dir, device_ids):
        import jax
        jax.devices()
        if device_ids:
            ids = (ctypes.c_int64 * len(device_ids))(*device_ids)
            rc = lib.axon_start_nrt_profile(ids, len(device_ids))
        else:
            rc = lib.axon_start_nrt_profile(None, 0)
        if rc != 0:
            raise RuntimeError(f"axon_start_nrt_profile rc={rc}")
        try:
            yield
        finally:
            n = lib.axon_stop_nrt_profile(str(output_dir).encode())
            print(f"profile: {n} file(s) written to {output_dir}", flush=True)

    return _hook


def _split_multi_waits(nc):
    """This container's walrus accepts only 1 sync wait per instruction:
    spill extra waits onto preceding wait-only NoOps."""
    for f in nc.m.functions:
        for bb in f.blocks:
            insts = bb.instructions
            if not any(i.sync_info is not None and len(i.sync_info.on_wait) > 1
                       for i in insts):
                continue
            out = []
            for inst in insts:
                si = inst.sync_info
                if si is not None and len(si.on_wait) > 1:
                    waits = list(si.on_wait)
                    for w in waits[:-1]:
                        _split_counter[0] += 1
                        nop = mybir.InstNoOp(
                            name=f"I-waitsplit-{_split_counter[0]}", ins=[], outs=[])
                        nop.engine = inst.engine
                        nop.sync_info = bass_rust.SyncInfo(on_wait=[w], on_update=[])
                        out.append(nop)
                    inst.sync_info = bass_rust.SyncInfo(
                        on_wait=waits[-1:], on_update=list(si.on_update))
                out.append(inst)
            bb.instructions = out


def _ldw_dedupe(nc):
    """Drop an InstLdweights identical to the previous one on PE (physical
    APs are per-tile-instance, so equality is collision-safe); carry its
    waits onto the next PE instruction. Saves ~90ns of weight-load
    serialization per duplicate (no ldw-opt / background buffer here)."""
    for f in nc.m.functions:
        for bb in f.blocks:
            insts = bb.instructions
            out = []
            last_sig = None
            pending = []
            dropped = 0
            for inst in insts:
                tn = type(inst).__name__
                if tn == 'InstLdweights':
                    sig = (repr(inst.ins[0]), repr(inst.tile_position),
                           repr(inst.perf_mode), repr(inst.is_transpose))
                    si = inst.sync_info
                    no_upd = si is None or len(si.on_update) == 0
                    if sig == last_sig and no_upd:
                        dropped += 1
                        if si is not None and len(si.on_wait) > 0:
                            pending.extend(si.on_wait)
                        continue
                    last_sig = sig
                elif tn == 'InstMatmult':
                    # a matmul whose stationary operand differs from the
                    # tracked LDW is self-loading (fp32/f32r) and clobbers
                    # the weight registers
                    if last_sig is None or \
                            (len(inst.ins) > 1 and repr(inst.ins[1]) != last_sig[0]):
                        last_sig = None
                else:
                    if getattr(inst, 'engine', None) is not None and \
                            str(inst.engine) == 'EngineType.PE':
                        last_sig = None
                if pending and getattr(inst, 'engine', None) is not None \
                        and str(inst.engine) == 'EngineType.PE':
                    si = inst.sync_info
                    ws = list(si.on_wait) if si else []
                    us = list(si.on_update) if si else []
                    inst.sync_info = bass_rust.SyncInfo(on_wait=pending + ws,
                                                        on_update=us)
                    pending = []
                out.append(inst)
            assert not pending
            if dropped:
                bb.instructions = out


def _install():
    if _installed[0]:
        return
    _installed[0] = True

    if 'antenv.axon_hooks' not in sys.modules:
        try:
            mod = types.ModuleType('antenv.axon_hooks')
            hook = _make_ntff_hook('/opt/axon/libaxon_pjrt.so')
            mod.get_axon_ntff_profile_hook = lambda: hook
            sys.modules['antenv.axon_hooks'] = mod
        except Exception:
            pass

    def patched_drain(self, tick_clock, wait_clock):
        from concourse.vector_clock import ScopedClock
        drain_inst = self.nc.sync.drain()
        wait_clock.add_sem_waits(drain_inst.ins,
                                 ScopedClock({None: tick_clock.global_clock}))
        inst = drain_inst.ins
        waits = list(inst.sync_info.on_wait)
        if len(waits) > 1:
            inst.sync_info = bass_rust.SyncInfo(on_wait=waits[:1], on_update=[])
            for i in range(1, len(waits)):
                d2 = self.nc.sync.drain()
                d2.ins.sync_info = bass_rust.SyncInfo(on_wait=waits[i:i + 1],
                                                      on_update=[])
        self.nc.all_engine_barrier()
        popped = self.nc._tile_sem_poison_stack.pop()
        assert popped is self._sem_poison
        self.nc.clear_and_free_semaphores(list(self.sems.allocated().values()))
        # the second all-engine barrier only orders the sem clears against
        # program end; Pool executes them before halting either way.

    tile.TileContext._drain_and_barrier = patched_drain

    orig_exit = tile.TileContext.__exit__

    def patched_exit(self, exc_type, exc_value, traceback):
        r = orig_exit(self, exc_type, exc_value, traceback)
        if exc_type is None:
            _ldw_dedupe(self.nc)
            _split_multi_waits(self.nc)
        return r

    tile.TileContext.__exit__ = patched_exit


def build_kernel(with_bias_rank1, with_qk_bias=False, fused_qk=True):
    scale_exp = SCALE_FUSED if fused_qk else SCALE_PLAIN
    nc = bass.Bass()
    xs = nc.declare_dram_parameter("xs", [NB, C, N], F32, isOutput=False)
    out_d = nc.declare_dram_parameter("out", [NB, C, N], F32, isOutput=True)
    # weights arrive pre-rearranged [128, 2, C] and already fp8 (host-side
    # cast): 64KB each, so they clear the DMA queues ~8x sooner and need no
    # on-chip conversion
    w_names = (["wuT"] if fused_qk else ["wqT", "wkT"]) + ["wovT"]
    w_d = {}
    for w in w_names:
        w_d[w] = nc.declare_dram_parameter(w, [128, NC2, C], FP8, isOutput=False)
    if with_qk_bias:
        bq_d = nc.declare_dram_parameter("bq", [C], F32, isOutput=False)
        bk_d = nc.declare_dram_parameter("bk", [C], F32, isOutput=False)
    if with_bias_rank1:
        Bf_d = nc.declare_dram_parameter("Bf", [C], F32, isOutput=False)
    gamma_d = nc.declare_dram_parameter("gamma", [C], F32, isOutput=False)
    beta_d = nc.declare_dram_parameter("beta", [C], F32, isOutput=False)
    sel_d = nc.declare_dram_parameter("sel128", [128, 128], F32, isOutput=False)

    with tile.TileContext(nc) as tc:
        ctx = contextlib.ExitStack()
        with ctx:
            consts = ctx.enter_context(tc.tile_pool(name="consts", bufs=1))
            xp = ctx.enter_context(tc.tile_pool(name="xp", bufs=2 * NB))
            sqp = ctx.enter_context(tc.tile_pool(name="sqp", bufs=2))
            hp = ctx.enter_context(tc.tile_pool(name="hp", bufs=8))
            qkp = ctx.enter_context(tc.tile_pool(name="qkp", bufs=6))
            vtp = ctx.enter_context(tc.tile_pool(name="vtp", bufs=3))
            pup = ctx.enter_context(tc.tile_pool(name="pup", bufs=8))
            finp = ctx.enter_context(tc.tile_pool(name="finp", bufs=6))
            smalls = ctx.enter_context(tc.tile_pool(name="smalls", bufs=16))
            rp = ctx.enter_context(tc.tile_pool(name="rp", bufs=4))
            # PSUM (8 banks): sT-pair ring "s" (2 x 2 banks), ou accumulator
            # (2 banks, per-ic lifetime), r (1 bank), proj scratch "p"
            # (1 bank; proj psums ping-pong through it between att matmuls).
            psp = ctx.enter_context(tc.tile_pool(name="psp", bufs=2, space="PSUM"))
            accp = ctx.enter_context(tc.tile_pool(name="accp", bufs=1, space="PSUM"))
            rpsp = ctx.enter_context(tc.tile_pool(name="rpsp", bufs=1, space="PSUM"))
            ppsp = ctx.enter_context(tc.tile_pool(name="ppsp", bufs=1, space="PSUM"))

            # PE warmup: start the cold IRAM fetch + HAM window immediately
            warm = consts.tile([1, 2], F32, tag="warm")
            nc.vector.memset(warm[:], 1.0)
            warm_ps = psp.tile([2, 2], F32, tag="s", name="warmps")
            nc.tensor.matmul(warm_ps[:], warm[:], warm[:], start=True, stop=True)
            # ACT warmup: every activation below resolves to the single
            # natural_log_exp table (Identity/Square/Exp/Ln); a dummy Exp up
            # front hoists its 1.3us load off the critical path
            warm_act = consts.tile([1, 2], F32, tag="warm_act")
            nc.scalar.activation(warm_act[:], warm[:], AF.Exp)

            # DMA order: x(0), tiny GN constants, WEIGHTS (needed ~9us in by
            # proj(0)), then x(1..3). Everything is queued up-front on the
            # sync engine; arrival order ~ issue order at ~170 GB/s.
            all_x = [[None, None] for _ in range(NB)]
            for t in range(NC2):
                x_t = xp.tile([128, N], F32, tag="x", name=f"x0_{t}")
                # split halves so bn_stats(0) can start on the first chunk
                for hh in range(2):
                    nc.sync.dma_start(
                        out=x_t[:, hh * 512:(hh + 1) * 512],
                        in_=xs[0, t * 128:(t + 1) * 128, hh * 512:(hh + 1) * 512])
                all_x[0][t] = x_t

            sel = consts.tile([128, 128], F32, tag="sel")
            nc.sync.dma_start(out=sel, in_=sel_d[:, :])
            gamma = consts.tile([128, NC2], F32, tag="gamma")
            nc.sync.dma_start(out=gamma, in_=gamma_d.rearrange("(t p) -> p t", p=128))
            beta = consts.tile([128, NC2], F32, tag="beta")
            nc.sync.dma_start(out=beta, in_=beta_d.rearrange("(t p) -> p t", p=128))

            wtiles = {}
            for wname in w_names:
                wt = consts.tile([128, NC2, C], FP8, tag=wname)
                nc.sync.dma_start(out=wt, in_=w_d[wname][:, :, :])
                wtiles[wname] = wt
            if fused_qk:
                wuT = wtiles["wuT"]
            else:
                wqT = wtiles["wqT"]
                wkT = wtiles["wkT"]
            wovT = wtiles["wovT"]

            bqv = bkv = None
            if with_qk_bias:
                bqv = consts.tile([128, NC2], F32, tag="bqv")
                nc.sync.dma_start(out=bqv, in_=bq_d.rearrange("(t p) -> p t", p=128))
                bkv = consts.tile([128, NC2], F32, tag="bkv")
                nc.sync.dma_start(out=bkv, in_=bk_d.rearrange("(t p) -> p t", p=128))
            if with_bias_rank1:
                Bfc = consts.tile([128, NC2], F32, tag="Bfc")
                nc.sync.dma_start(out=Bfc, in_=Bf_d.rearrange("(t p) -> p t", p=128))

            for b in range(1, NB):
                for t in range(NC2):
                    x_t = xp.tile([128, N], F32, tag="x", name=f"x{b}_{t}")
                    nc.sync.dma_start(out=x_t, in_=xs[b, t * 128:(t + 1) * 128, :])
                    all_x[b][t] = x_t

            # tiny constants computed on-chip (no DMA dependency)
            eps_t = consts.tile([128, 1], F32, tag="eps")
            nc.vector.memset(eps_t[:], EPS)
            mln16 = consts.tile([128, 1], F32, tag="mln16")
            nc.vector.memset(mln16[:], -math.log(WSCALE))
            ones128 = consts.tile([128, 2, 128], FP8, tag="ones128")
            nc.vector.memset(ones128[:], 1.0)

            S = [dict() for _ in range(NB)]
            for b in range(NB):
                S[b]["x"] = all_x[b]

            def gn_a1(b):
                """bn_stats passes (emitted early so they run on the idle DVE
                while the previous att half streams)."""
                st_ = S[b]
                xt = st_["x"]
                stats = smalls.tile([128, NC2, 2], F32, tag="gnstats",
                                    name=f"gst{b}")
                bsts = [None, None]
                for t in range(NC2):
                    # all-DVE: the DVE engine is live at t~0 while ACT (and
                    # PE) pay a ~7.8us engine-init tax, so even sample 0's
                    # serial GN chain is fastest kept entirely on DVE
                    bst = sqp.tile([128, 2, nc.vector.BN_STATS_DIM], F32,
                                   tag="bst", name=f"bst{b}_{t}")
                    for sub in range(2):
                        nc.vector.bn_stats(out=bst[:, sub, :],
                                           in_=xt[t][:, sub * 512:(sub + 1) * 512])
                    bsts[t] = bst
                st_["stats"] = stats
                st_["bsts"] = bsts

            def gn_a2(b):
                """aggregate bn stats -> [mean, E[x^2]] per partition."""
                st_ = S[b]
                stats = st_["stats"]
                for t in range(NC2):
                    bst = st_["bsts"][t]
                    if bst is None:
                        continue
                    mv = stats[:, t, :]
                    nc.vector.bn_aggr(out=mv, in_=bst[:])
                    # mv = [mean, var] -> [mean, E[x^2]]
                    msq = smalls.tile([128, 1], F32, tag="msq", name=f"msq{b}_{t}")
                    nc.vector.tensor_mul(msq[:], mv[:, 0:1], mv[:, 0:1])
                    nc.vector.tensor_add(mv[:, 1:2], mv[:, 1:2], msq[:])

            def gn_mid(b):
                """one block-diag selector matmul -> per-channel group stats
                (already broadcast), then [128, t] affine columns."""
                st_ = S[b]
                gs_ps = ppsp.tile([128, NC2, 2], F32, tag="p", name=f"gsps{b}")
                # sel is host-scaled by 1/32, so gs_ps holds the per-group
                # [mean, E[x^2]] directly and DVE reads it straight from PSUM
                # (no ACT drain hop on this serial chain)
                nc.tensor.matmul(gs_ps.rearrange("p t s -> p (t s)"),
                                 sel[:, :],
                                 st_["stats"].rearrange("p t s -> p (t s)"),
                                 start=True, stop=True)
                # one contiguous DVE drain (strided PSUM reads are rejected
                # by the BIR verifier), then tiny SBUF ops
                gsb = smalls.tile([128, NC2, 2], F32, tag="gsb", name=f"gsb{b}")
                nc.vector.tensor_copy(gsb.rearrange("p t s -> p (t s)"),
                                      gs_ps.rearrange("p t s -> p (t s)"))
                mm2 = smalls.tile([128, NC2], F32, tag="mm2", name=f"mm2{b}")
                nc.vector.tensor_mul(mm2[:], gsb[:, :, 0], gsb[:, :, 0])
                var = smalls.tile([128, NC2], F32, tag="var", name=f"var{b}")
                nc.vector.tensor_sub(var[:], gsb[:, :, 1], mm2[:])
                # rsqrt as exp(-0.5 ln(var+eps)): stays on the one loaded
                # Exp/Ln ACT table (Sqrt would force a table swap per sample)
                nc.scalar.activation(var[:], var[:], AF.Ln, bias=eps_t[:])
                a_cols = smalls.tile([128, NC2], F32, tag="acol", name=f"ac{b}")
                nc.scalar.activation(a_cols[:], var[:], AF.Exp, scale=-0.5)
                nc.vector.tensor_mul(a_cols[:], a_cols[:], gamma[:])
                c_cols = smalls.tile([128, NC2], F32, tag="ccol", name=f"cc{b}")
                nc.vector.tensor_mul(c_cols[:], gsb[:, :, 0], a_cols[:])
                nc.vector.tensor_sub(c_cols[:], beta[:], c_cols[:])
                st_["cols"] = (a_cols, c_cols)

            def gn_b(b):
                """h = a*x + c, written into a single [128, 2, N] fp8 tile
                (the layout DoubleRow matmuls contract K=256 over)."""
                st_ = S[b]
                a_cols, c_cols = st_["cols"]
                h2 = hp.tile([128, NC2, N], FP8, tag="h", name=f"h{b}")
                # half-width ops, h-half outer: the u projection's first
                # 512-chunk unblocks after only two of the four ops.  Sample
                # 0 puts the t=1 halves on the (initialized, idle) ACT.
                for hh in range(2):
                    hsl = slice(hh * 512, (hh + 1) * 512)
                    for t in range(NC2):
                        if b == 0 and t == 1:
                            nc.scalar.activation(h2[:, t, hsl],
                                                 st_["x"][t][:, hsl],
                                                 AF.Identity,
                                                 bias=c_cols[:, t:t + 1],
                                                 scale=a_cols[:, t:t + 1])
                        else:
                            nc.vector.tensor_scalar(h2[:, t, hsl],
                                                    st_["x"][t][:, hsl],
                                                    a_cols[:, t:t + 1],
                                                    c_cols[:, t:t + 1],
                                                    op0=OP.mult, op1=OP.add)
                st_["h2"] = h2

            def proj_pieces(b, ring, ring_tag, ring_v=None, tag_v=None):
                if ring_v is None:
                    ring_v, tag_v = ring, ring_tag
                """u (fused qk) or q,k, plus M = (wov h)^T, all fp8 — as a
                list of zero-arg emit callables so the pieces can be
                interleaved between att matmuls (each piece ping-pongs one
                1-bank PSUM slot against its DVE drain copy)."""
                st_ = S[b]
                pieces = []

                def qk_piece(wT, bias_cols, qktag, dst, mt, icc):
                    def go():
                        h2 = st_["h2"]   # set by gn_b, possibly a later piece
                        osl = slice(icc * 512, (icc + 1) * 512)
                        ps = ring.tile([128, 512], F32, tag=ring_tag,
                                       name=f"{qktag}ps{b}_{mt}_{icc}")
                        nc.tensor.matmul(
                            ps[:], wT[:, :, mt * 128:(mt + 1) * 128],
                            h2[:, :, osl], perf_mode=DR,
                            start=True, stop=True)
                        if bias_cols is None:
                            # tensor_copy hits the DVE copy perf mode
                            nc.vector.tensor_copy(dst[:, mt, osl], ps[:])
                        else:
                            nc.vector.tensor_scalar_add(
                                dst[:, mt, osl], ps[:], bias_cols[:, mt:mt + 1])
                    return go

                def qk(wT, bias_cols, qktag):
                    dst = qkp.tile([128, NC2, N], FP8, tag=qktag, name=f"{qktag}{b}")
                    # icc outer: the first att half (i < 512) only needs the
                    # icc=0 pieces of both mt chunks
                    for icc in range(NIC):
                        for mt in range(NC2):
                            pieces.append(qk_piece(wT, bias_cols, qktag,
                                                   dst, mt, icc))
                    return dst

                if fused_qk:
                    st_["mov"] = qk(wuT, None, "u")
                    st_["sta"] = None   # resolved to h2 in att_h
                else:
                    st_["mov"] = qk(wqT, bqv if with_qk_bias else None, "u")
                    st_["sta"] = qk(wkT, bkv if with_qk_bias else None, "k")
                # M in fp8 [128, jpair, 2, C]; two j's share one PSUM tile so
                # the drain is a single 512-wide DVE copy per pair
                vT = vtp.tile([128, NJ // 2, 2, C], FP8, tag="vt", name=f"vt{b}")

                def vt_piece(jp):
                    def go():
                        h2 = st_["h2"]
                        ps = ring_v.tile([128, 2, C], F32, tag=tag_v,
                                         name=f"vtps{b}_{jp}")
                        for s in range(2):
                            j = 2 * jp + s
                            nc.tensor.matmul(ps[:, s, :],
                                             h2[:, :, j * 128:(j + 1) * 128],
                                             wovT[:, :, :], perf_mode=DR,
                                             start=True, stop=True)
                        nc.vector.tensor_copy(
                            vT[:, jp, :, :].rearrange("p s c -> p (s c)"),
                            ps.rearrange("p s c -> p (s c)"))
                    return go

                for jp in range(NJ // 2):
                    pieces.append(vt_piece(jp))
                st_["vT"] = vT
                return pieces

            NP = NJ // 2

            def att_h(b, ic, pieces=(), schedule=(), pre_sp=None,
                      prefetch_next=False):
                """one i-half of attention: scores^T pairs -> single
                1024-wide exp -> output accumulation, then the broadcast row
                sums in one LdW-deduped ones-matmul chain.  `pieces` are
                gn/proj emit callables slotted after each jp's matmuls
                (schedule[jp] = how many); a piece at slot jp must only
                produce data consumed at jp+1 or later.  Leftovers are
                returned for the caller to place.  With prefetch_next, the
                NEXT half's first sT pair is emitted into the exp(jp3) wait
                window before the row-sum chain (it has no exp dependency)
                and handed back for that half to consume via pre_sp."""
                pieces = list(pieces)
                st_ = S[b]
                sta, mov, vT = st_["sta"], st_["mov"], st_["vT"]
                if sta is None:
                    sta = st_["h2"]
                isl = slice(ic * 512, (ic + 1) * 512)
                ou = accp.tile([128, NC2, 512], F32, tag="ou",
                               name=f"ou{b}_{ic}")
                r_ps = rpsp.tile([128, 512], F32, tag="r", name=f"rps{b}_{ic}")

                def emit_pair(jp, picl=ic):
                    isl_ = slice(picl * 512, (picl + 1) * 512)
                    sp = psp.tile([128, 2, 512], F32, tag="s",
                                  name=f"sT{b}_{picl}_{jp}")
                    for s2 in range(2):
                        j = 2 * jp + s2
                        nc.tensor.matmul(sp[:, s2, :],
                                         sta[:, :, j * 128:(j + 1) * 128],
                                         mov[:, :, isl_], perf_mode=DR,
                                         start=True, stop=True)
                    return sp

                tail = (b == NB - 1 and ic == 1)
                sched = dict(schedule)
                sp = pre_sp if pre_sp is not None else emit_pair(0)
                pu2s = []

                def emit_rsums():
                    # back-to-back so the LdW dedupe keeps one ones
                    # weight-load per half
                    for jp in range(NP):
                        nc.tensor.matmul(r_ps[:], ones128[:], pu2s[jp][:],
                                         perf_mode=DR,
                                         start=(jp == 0), stop=(jp == NP - 1))

                for jp in range(NP):
                    pu2 = pup.tile([128, 2, 512], FP8, tag="pu",
                                   name=f"pu{b}_{ic}_{jp}")
                    nc.scalar.activation(pu2.rearrange("p s i -> p (s i)"),
                                         sp.rearrange("p s i -> p (s i)"),
                                         AF.Exp, scale=scale_exp)
                    pu2s.append(pu2)
                    if jp + 1 < NP:
                        sp = emit_pair(jp + 1)
                    if tail and jp == NP - 1:
                        # final half: row sums first so epi's ln/exp overlap
                        # the remaining output matmuls
                        emit_rsums()
                    for ct in range(NC2):
                        nc.tensor.matmul(ou[:, ct, :],
                                         vT[:, jp, :, ct * 128:(ct + 1) * 128],
                                         pu2[:], perf_mode=DR,
                                         start=(jp == 0), stop=(jp == NP - 1))
                    for _ in range(sched.get(jp, 0)):
                        if pieces:
                            pieces.pop(0)()
                next_sp = emit_pair(0, ic + 1) if prefetch_next else None
                if not tail:
                    emit_rsums()
                st_[f"ou{ic}"] = ou
                st_[f"r{ic}"] = r_ps
                return pieces, next_sp

            def epi_h(b, ic):
                """1/(16 r) = exp(-ln r - ln 16) on the broadcast row sums,
                then normalize, residual, store for this i-half."""
                st_ = S[b]
                ou, r_ps, xt = st_[f"ou{ic}"], st_[f"r{ic}"], st_["x"]
                isl = slice(ic * 512, (ic + 1) * 512)
                lnr = rp.tile([128, 512], F32, tag="lnr", name=f"lnr{b}_{ic}")
                rinv = rp.tile([128, 512], F32, tag="rinv", name=f"rinv{b}_{ic}")
                nc.scalar.activation(lnr[:], r_ps[:], AF.Ln)
                # bias: wov is host-scaled by WSCALE -> fold 1/WSCALE here
                nc.scalar.activation(rinv[:], lnr[:], AF.Exp, scale=-1.0,
                                     bias=mln16[:])
                last = (b == NB - 1)
                for ct in range(NC2):
                    fin = finp.tile([128, 512], F32, tag="fin",
                                    name=f"fin{b}_{ic}_{ct}")
                    src = ou[:, ct, :]
                    if with_bias_rank1:
                        # ou += Bf[o] * r[i] as a fused DVE op on the
                        # broadcast row sums (scores bias rank-1 term)
                        srcb = finp.tile([128, 512], F32, tag="finb",
                                         name=f"finb{b}_{ic}_{ct}")
                        nc.vector.scalar_tensor_tensor(
                            srcb[:], r_ps[:], Bfc[:, ct:ct + 1], src,
                            op0=OP.mult, op1=OP.add)
                        src = srcb[:]
                    nc.vector.tensor_tensor(fin[:], src, rinv[:], op=OP.mult)
                    if last:
                        # tail latency: keep the residual add on DVE
                        nc.vector.tensor_add(fin[:], fin[:], xt[ct][:, isl])
                    else:
                        # gpsimd add overlaps mid-kernel
                        nc.gpsimd.tensor_add(fin[:], fin[:], xt[ct][:, isl])
                    nc.sync.dma_start(out=out_d[b, ct * 128:(ct + 1) * 128, isl],
                                      in_=fin[:])

            # ---- pipelined emission ----
            # ic-outer halves.  Sample b+1's work drips into sample b's
            # stream as interleaved pieces: bn_stats before epi_h(b,0) (DVE
            # idles during the first half), the rest of GN + the u
            # projection inside att(b,1)'s jp slots, and the vT pieces carry
            # over into att(b+1,0)'s early slots (vT[jp] is consumed one
            # slot later than it is produced).
            gn_a1(0)
            gn_a2(0)
            gn_mid(0)
            gn_b(0)
            # startup proj: interleave u pieces (2-slot "s" ring) with vT
            # pieces ("p" ring) so the two psum slots ping-pong in parallel
            p0 = proj_pieces(0, psp, "s", ppsp, "p")
            nu = len(p0) - NJ // 2
            for i in range(max(nu, NJ // 2)):
                if i < nu:
                    p0[i]()
                if i < NJ // 2:
                    p0[nu + i]()
            carry = []
            for b in range(NB):
                carry, sp1 = att_h(b, 0, carry, {0: 1, 1: 1, 2: 1, 3: 99},
                                   prefetch_next=True)
                if b + 1 < NB:
                    # bn_stats + aggregation run on the idle DVE while this
                    # half's matmul/exp stream is in flight
                    gn_a1(b + 1)
                    gn_a2(b + 1)
                epi_h(b, 0)
                if b + 1 < NB:
                    bn = b + 1

                    def gn_piece1(bn=bn):
                        gn_mid(bn)

                    def gn_piece2(bn=bn):
                        gn_b(bn)

                    pieces = [gn_piece1, gn_piece2]
                    pieces += proj_pieces(b + 1, ppsp, "p")
                else:
                    pieces = []
                carry, _ = att_h(b, 1, pieces, {0: 2, 1: 2, 2: 2, 3: 3},
                                 pre_sp=sp1)
                epi_h(b, 1)
            for pc in carry:
                pc()

    return nc


_cache = {}


def kernel(x, gamma, beta, wq, bq, wk, bk, wv, bv, wo, bo):
    """Full inputs -> full output. Shards batch 4/core over 8 cores."""
    _install()
    from concourse.bass_utils import run_bass_kernel_spmd

    x = np.asarray(x)
    B, Cc, H, W = x.shape
    assert (Cc, H * W) == (C, N) and B == NB * NCORES
    xf = np.ascontiguousarray(x.reshape(B, C, N).astype(np.float32))

    wq = np.asarray(wq); wk = np.asarray(wk); wv = np.asarray(wv); wo = np.asarray(wo)
    bq = np.asarray(bq); bk = np.asarray(bk); bv = np.asarray(bv); bo = np.asarray(bo)
    gamma = np.asarray(gamma); beta = np.asarray(beta)

    Bf = (wo.astype(np.float64) @ bv.astype(np.float64) + bo).astype(np.float32)
    wov = (wo.astype(np.float64) @ wv.astype(np.float64)) * WSCALE
    has_bias = bool(np.any(Bf != 0.0))
    has_qk_bias = bool(np.any(bq != 0.0) or np.any(bk != 0.0))
    fused_qk = not has_qk_bias

    # block-diag group selector pre-scaled by 1/32 (channels per group) so
    # the group-sum matmul yields means directly
    sel128 = np.zeros((128, 128), np.float32)
    for p in range(128):
        g0 = (p // 32) * 32
        sel128[p, g0:g0 + 32] = 1.0 / 32.0

    import ml_dtypes
    FP8NP = ml_dtypes.float8_e4m3

    def stage_w(wT):
        # [C(a), C(b)] -> [128, 2, C] fp8: a = (t p) split across tile dim
        return np.ascontiguousarray(
            wT.astype(np.float32).reshape(NC2, 128, C).transpose(1, 0, 2)
        ).astype(FP8NP)

    common = {
        "wovT": stage_w(wov.T),
        "gamma": gamma.astype(np.float32), "beta": beta.astype(np.float32),
        "sel128": sel128,
    }
    if fused_qk:
        wu = (wq.astype(np.float64).T @ wk.astype(np.float64)) * WSCALE
        common["wuT"] = stage_w(wu)
    else:
        common["wqT"] = stage_w(wq.T)
        common["wkT"] = stage_w(wk.T)
        common["bq"] = bq.astype(np.float32)
        common["bk"] = bk.astype(np.float32)
    if has_bias:
        # the rank-1 term rides through the 1/WSCALE folded into rinv
        common["Bf"] = Bf * WSCALE
    in_maps = []
    for c in range(NCORES):
        m = dict(common)
        m["xs"] = np.ascontiguousarray(xf[c * NB:(c + 1) * NB])
        in_maps.append(m)

    key = (has_bias, has_qk_bias)
    if key not in _cache:
        _cache[key] = build_kernel(with_bias_rank1=has_bias,
                                   with_qk_bias=has_qk_bias,
                                   fused_qk=fused_qk)
    nc = _cache[key]

    trace = os.environ.get("TRN_KERNEL_TRACE", "0") == "1"
    kw = {}
    if trace:
        import shutil, tempfile
        td = os.environ.get("TRN_KERNEL_TRACE_DIR") or tempfile.mkdtemp()
        shutil.rmtree(td, ignore_errors=True)
        os.makedirs(td, exist_ok=True)
        kw = dict(trace=True, tmpdir=td)
    res = run_bass_kernel_spmd(nc, in_maps, list(range(NCORES)), **kw)
    _last_exec_time_ns[0] = getattr(res, "exec_time_ns", None)

    full = np.concatenate([res.results[c]["out"] for c in range(NCORES)], axis=0)
    return full.reshape(B, C, H, W).astype(np.float32)


def last_exec_time_ns():
    return _last_exec_time_ns[0]


# revision 59
# speedup vs baseline: 1.1654x; 1.1654x over previous
"""Trainium2 Bass kernel for nn_AttentionBlock (GroupNorm + spatial
self-attention + residual), data-parallel over batch across 8 NeuronCores.

Self-contained: patches the container's concourse runtime (walrus here only
accepts 1 sync wait per instruction; LDWEIGHTS dedupe; optional NTFF
profiling), builds the Tile kernel, shards inputs 4 samples/core, runs SPMD
on cores 0-7, gathers the full output.

Math per sample (x: [C=256, N=1024]):
  h  = GN_8groups(x) * gamma + beta                    [C, N]
  u  = (16 wq^T wk)^T h            (qk fused: s = q.k = h^T (wk^T wq) h)
  M  = ((16 wo wv) h)^T                                [N, C]  (out-proj fused)
  sT[j,i] = sum_b h[b,j] u[b,i]                        (j on partitions)
  Pu = exp(sT/256)         (scores are tiny; no max subtraction needed)
  ou[o,i] = sum_j M[j,o] Pu[j,i]
  r_bc[p,i] = sum_j Pu[j,i]  (ones-matmul with 128 ones columns ->
                              the row-sum lands pre-broadcast on all
                              128 partitions; no separate bcast matmul)
  rinv = exp(-ln r - ln 16)            (fast ACT tables; folds the 1/16)
  out = x + ou * rinv_bc

GroupNorm group sums use one block-diagonal [128,128] selector matmul that
returns the per-group sums already broadcast per-channel. All large matmuls
run in bf16/fp8 (fp32 accumulate in PSUM); GroupNorm statistics and the
final combine are fp32. Emission is software-pipelined across the 4 samples
so GroupNorm/projection chains hide under attention matmuls.
"""
import contextlib
import ctypes
import math
import os
import sys
import types

sys.path.insert(0, '/opt/trn_rl_repo')

import numpy as np

import bass_rust
import concourse.bass as bass
import concourse.tile as tile
from concourse import mybir

F32 = mybir.dt.float32
F32R = mybir.dt.float32r
BF16 = mybir.dt.bfloat16
FP8 = mybir.dt.float8e4
DR = mybir.MatmulPerfMode.DoubleRow
AX = mybir.AxisListType.X
AF = mybir.ActivationFunctionType
OP = mybir.AluOpType

C = 256
N = 1024
G = 8
EPS = 1e-5
WSCALE = 16.0          # fp8-range scaling folded into wu / wov
SCALE_FUSED = 1.0 / (16.0 * WSCALE)   # exp scale, fused-qk build
SCALE_PLAIN = 1.0 / 16.0              # exp scale, separate-q/k build
NB = 4   # samples per core
NCORES = 8
NC2 = C // 128
NJ = N // 128
NIC = N // 512

_installed = [False]
_split_counter = [0]
_last_exec_time_ns = [None]


def _make_ntff_hook(so_path):
    lib = ctypes.CDLL(so_path)
    lib.axon_start_nrt_profile.argtypes = [ctypes.POINTER(ctypes.c_int64), ctypes.c_size_t]
    lib.axon_start_nrt_profile.restype = ctypes.c_int64
    lib.axon_stop_nrt_profile.argtypes = [ctypes.c_char_p]
    lib.axon_stop_nrt_profile.restype = ctypes.c_int64

    @contextlib.contextmanager
    def _hook(output_dir, device_ids):
        import jax
        jax.devices()
        if device_ids:
            ids = (ctypes.c_int64 * len(device_ids))(*device_ids)
            rc = lib.axon_start_nrt_profile(ids, len(device_ids))
        else:
            rc = lib.axon_start_nrt_profile(None, 0)
        if rc != 0:
            raise RuntimeError(f"axon_start_nrt_profile rc={rc}")
        try:
            yield
        finally:
            n = lib.axon_stop_nrt_profile(str(output_dir).encode())
            print(f"profile: {n} file(s) written to {output_dir}", flush=True)

    return _hook


def _split_multi_waits(nc):
    """This container's walrus accepts only 1 sync wait per instruction:
    spill extra waits onto preceding wait-only NoOps."""
    for f in nc.m.functions:
        for bb in f.blocks:
            insts = bb.instructions
            if not any(i.sync_info is not None and len(i.sync_info.on_wait) > 1
                       for i in insts):
                continue
            out = []
            for inst in insts:
                si = inst.sync_info
                if si is not None and len(si.on_wait) > 1:
                    waits = list(si.on_wait)
                    for w in waits[:-1]:
                        _split_counter[0] += 1
                        nop = mybir.InstNoOp(
                            name=f"I-waitsplit-{_split_counter[0]}", ins=[], outs=[])
                        nop.engine = inst.engine
                        nop.sync_info = bass_rust.SyncInfo(on_wait=[w], on_update=[])
                        out.append(nop)
                    inst.sync_info = bass_rust.SyncInfo(
                        on_wait=waits[-1:], on_update=list(si.on_update))
                out.append(inst)
            bb.instructions = out


def _ldw_dedupe(nc):
    """Drop an InstLdweights identical to the previous one on PE (physical
    APs are per-tile-instance, so equality is collision-safe); carry its
    waits onto the next PE instruction. Saves ~90ns of weight-load
    serialization per duplicate (no ldw-opt / background buffer here)."""
    for f in nc.m.functions:
        for bb in f.blocks:
            insts = bb.instructions
            out = []
            last_sig = None
            pending = []
            dropped = 0
            for inst in insts:
                tn = type(inst).__name__
                if tn == 'InstLdweights':
                    sig = (repr(inst.ins[0]), repr(inst.tile_position),
                           repr(inst.perf_mode), repr(inst.is_transpose))
                    si = inst.sync_info
                    no_upd = si is None or len(si.on_update) == 0
                    if sig == last_sig and no_upd:
                        dropped += 1
                        if si is not None and len(si.on_wait) > 0:
                            pending.extend(si.on_wait)
                        continue
                    last_sig = sig
                elif tn == 'InstMatmult':
                    # a matmul whose stationary operand differs from the
                    # tracked LDW is self-loading (fp32/f32r) and clobbers
                    # the weight registers
                    if last_sig is None or \
                            (len(inst.ins) > 1 and repr(inst.ins[1]) != last_sig[0]):
                        last_sig = None
                else:
                    if getattr(inst, 'engine', None) is not None and \
                            str(inst.engine) == 'EngineType.PE':
                        last_sig = None
                if pending and getattr(inst, 'engine', None) is not None \
                        and str(inst.engine) == 'EngineType.PE':
                    si = inst.sync_info
                    ws = list(si.on_wait) if si else []
                    us = list(si.on_update) if si else []
                    inst.sync_info = bass_rust.SyncInfo(on_wait=pending + ws,
                                                        on_update=us)
                    pending = []
                out.append(inst)
            assert not pending
            if dropped:
                bb.instructions = out


def _install():
    if _installed[0]:
        return
    _installed[0] = True

    if 'antenv.axon_hooks' not in sys.modules:
        try:
            mod = types.ModuleType('antenv.axon_hooks')
            hook = _make_ntff_hook('/opt/axon/libaxon_pjrt.so')
            mod.get_axon_ntff_profile_hook = lambda: hook
            sys.modules['antenv.axon_hooks'] = mod
        except Exception:
            pass

    def patched_drain(self, tick_clock, wait_clock):
        from concourse.vector_clock import ScopedClock
        drain_inst = self.nc.sync.drain()
        wait_clock.add_sem_waits(drain_inst.ins,
                                 ScopedClock({None: tick_clock.global_clock}))
        inst = drain_inst.ins
        waits = list(inst.sync_info.on_wait)
        if len(waits) > 1:
            inst.sync_info = bass_rust.SyncInfo(on_wait=waits[:1], on_update=[])
            for i in range(1, len(waits)):
                d2 = self.nc.sync.drain()
                d2.ins.sync_info = bass_rust.SyncInfo(on_wait=waits[i:i + 1],
                                                      on_update=[])
        self.nc.all_engine_barrier()
        popped = self.nc._tile_sem_poison_stack.pop()
        assert popped is self._sem_poison
        self.nc.clear_and_free_semaphores(list(self.sems.allocated().values()))
        # the second all-engine barrier only orders the sem clears against
        # program end; Pool executes them before halting either way.

    tile.TileContext._drain_and_barrier = patched_drain

    orig_exit = tile.TileContext.__exit__

    def patched_exit(self, exc_type, exc_value, traceback):
        r = orig_exit(self, exc_type, exc_value, traceback)
        if exc_type is None:
            _ldw_dedupe(self.nc)
            _split_multi_waits(self.nc)
        return r

    tile.TileContext.__exit__ = patched_exit


def build_kernel(with_bias_rank1, with_qk_bias=False, fused_qk=True):
    scale_exp = SCALE_FUSED if fused_qk else SCALE_PLAIN
    nc = bass.Bass()
    xs = nc.declare_dram_parameter("xs", [NB, C, N], F32, isOutput=False)
    out_d = nc.declare_dram_parameter("out", [NB, C, N], F32, isOutput=True)
    # weights arrive pre-rearranged [128, 2, C] and already fp8 (host-side
    # cast): 64KB each, so they clear the DMA queues ~8x sooner and need no
    # on-chip conversion
    w_names = (["wuT"] if fused_qk else ["wqT", "wkT"]) + ["wovT"]
    w_d = {}
    for w in w_names:
        w_d[w] = nc.declare_dram_parameter(w, [128, NC2, C], FP8, isOutput=False)
    if with_qk_bias:
        bq_d = nc.declare_dram_parameter("bq", [C], F32, isOutput=False)
        bk_d = nc.declare_dram_parameter("bk", [C], F32, isOutput=False)
    if with_bias_rank1:
        Bf_d = nc.declare_dram_parameter("Bf", [C], F32, isOutput=False)
    gamma_d = nc.declare_dram_parameter("gamma", [C], F32, isOutput=False)
    beta_d = nc.declare_dram_parameter("beta", [C], F32, isOutput=False)
    sel_d = nc.declare_dram_parameter("sel128", [128, 128], F32, isOutput=False)

    with tile.TileContext(nc) as tc:
        ctx = contextlib.ExitStack()
        with ctx:
            consts = ctx.enter_context(tc.tile_pool(name="consts", bufs=1))
            xp = ctx.enter_context(tc.tile_pool(name="xp", bufs=2 * NB))
            sqp = ctx.enter_context(tc.tile_pool(name="sqp", bufs=2))
            hp = ctx.enter_context(tc.tile_pool(name="hp", bufs=8))
            qkp = ctx.enter_context(tc.tile_pool(name="qkp", bufs=6))
            vtp = ctx.enter_context(tc.tile_pool(name="vtp", bufs=3))
            pup = ctx.enter_context(tc.tile_pool(name="pup", bufs=8))
            finp = ctx.enter_context(tc.tile_pool(name="finp", bufs=6))
            smalls = ctx.enter_context(tc.tile_pool(name="smalls", bufs=16))
            rp = ctx.enter_context(tc.tile_pool(name="rp", bufs=4))
            # PSUM (8 banks): sT-pair ring "s" (2 x 2 banks), ou accumulator
            # (2 banks, per-ic lifetime), r (1 bank), proj scratch "p"
            # (1 bank; proj psums ping-pong through it between att matmuls).
            psp = ctx.enter_context(tc.tile_pool(name="psp", bufs=2, space="PSUM"))
            accp = ctx.enter_context(tc.tile_pool(name="accp", bufs=1, space="PSUM"))
            rpsp = ctx.enter_context(tc.tile_pool(name="rpsp", bufs=1, space="PSUM"))
            ppsp = ctx.enter_context(tc.tile_pool(name="ppsp", bufs=1, space="PSUM"))

            # PE warmup: start the cold IRAM fetch + HAM window immediately
            warm = consts.tile([1, 2], F32, tag="warm")
            nc.vector.memset(warm[:], 1.0)
            warm_ps = psp.tile([2, 2], F32, tag="s", name="warmps")
            nc.tensor.matmul(warm_ps[:], warm[:], warm[:], start=True, stop=True)
            # ACT warmup: every activation below resolves to the single
            # natural_log_exp table (Identity/Square/Exp/Ln); a dummy Exp up
            # front hoists its 1.3us load off the critical path
            warm_act = consts.tile([1, 2], F32, tag="warm_act")
            nc.scalar.activation(warm_act[:], warm[:], AF.Exp)

            # DMA order: x(0), tiny GN constants, WEIGHTS (needed ~9us in by
            # proj(0)), then x(1..3). Everything is queued up-front on the
            # sync engine; arrival order ~ issue order at ~170 GB/s.
            all_x = [[None, None] for _ in range(NB)]
            for t in range(NC2):
                x_t = xp.tile([128, N], F32, tag="x", name=f"x0_{t}")
                # split halves so bn_stats(0) can start on the first chunk
                for hh in range(2):
                    nc.sync.dma_start(
                        out=x_t[:, hh * 512:(hh + 1) * 512],
                        in_=xs[0, t * 128:(t + 1) * 128, hh * 512:(hh + 1) * 512])
                all_x[0][t] = x_t

            sel = consts.tile([128, 128], F32, tag="sel")
            nc.sync.dma_start(out=sel, in_=sel_d[:, :])
            gamma = consts.tile([128, NC2], F32, tag="gamma")
            nc.sync.dma_start(out=gamma, in_=gamma_d.rearrange("(t p) -> p t", p=128))
            beta = consts.tile([128, NC2], F32, tag="beta")
            nc.sync.dma_start(out=beta, in_=beta_d.rearrange("(t p) -> p t", p=128))

            wtiles = {}
            for wname in w_names:
                wt = consts.tile([128, NC2, C], FP8, tag=wname)
                nc.sync.dma_start(out=wt, in_=w_d[wname][:, :, :])
                wtiles[wname] = wt
            if fused_qk:
                wuT = wtiles["wuT"]
            else:
                wqT = wtiles["wqT"]
                wkT = wtiles["wkT"]
            wovT = wtiles["wovT"]

            bqv = bkv = None
            if with_qk_bias:
                bqv = consts.tile([128, NC2], F32, tag="bqv")
                nc.sync.dma_start(out=bqv, in_=bq_d.rearrange("(t p) -> p t", p=128))
                bkv = consts.tile([128, NC2], F32, tag="bkv")
                nc.sync.dma_start(out=bkv, in_=bk_d.rearrange("(t p) -> p t", p=128))
            if with_bias_rank1:
                Bfc = consts.tile([128, NC2], F32, tag="Bfc")
                nc.sync.dma_start(out=Bfc, in_=Bf_d.rearrange("(t p) -> p t", p=128))

            for b in range(1, NB):
                for t in range(NC2):
                    x_t = xp.tile([128, N], F32, tag="x", name=f"x{b}_{t}")
                    nc.sync.dma_start(out=x_t, in_=xs[b, t * 128:(t + 1) * 128, :])
                    all_x[b][t] = x_t

            # tiny constants computed on-chip (no DMA dependency)
            eps_t = consts.tile([128, 1], F32, tag="eps")
            nc.vector.memset(eps_t[:], EPS)
            mln16 = consts.tile([128, 1], F32, tag="mln16")
            nc.vector.memset(mln16[:], -math.log(WSCALE))
            ones128 = consts.tile([128, 2, 128], FP8, tag="ones128")
            nc.vector.memset(ones128[:], 1.0)

            S = [dict() for _ in range(NB)]
            for b in range(NB):
                S[b]["x"] = all_x[b]

            def gn_a1(b):
                """bn_stats passes (emitted early so they run on the idle DVE
                while the previous att half streams)."""
                st_ = S[b]
                xt = st_["x"]
                stats = smalls.tile([128, NC2, 2], F32, tag="gnstats",
                                    name=f"gst{b}")
                bsts = [None, None]
                for t in range(NC2):
                    # all-DVE: the DVE engine is live at t~0 while ACT (and
                    # PE) pay a ~7.8us engine-init tax, so even sample 0's
                    # serial GN chain is fastest kept entirely on DVE
                    bst = sqp.tile([128, 2, nc.vector.BN_STATS_DIM], F32,
                                   tag="bst", name=f"bst{b}_{t}")
                    for sub in range(2):
                        nc.vector.bn_stats(out=bst[:, sub, :],
                                           in_=xt[t][:, sub * 512:(sub + 1) * 512])
                    bsts[t] = bst
                st_["stats"] = stats
                st_["bsts"] = bsts

            def gn_a2(b):
                """aggregate bn stats -> [mean, E[x^2]] per partition."""
                st_ = S[b]
                stats = st_["stats"]
                for t in range(NC2):
                    bst = st_["bsts"][t]
                    if bst is None:
                        continue
                    mv = stats[:, t, :]
                    nc.vector.bn_aggr(out=mv, in_=bst[:])
                    # mv = [mean, var] -> [mean, E[x^2]]
                    msq = smalls.tile([128, 1], F32, tag="msq", name=f"msq{b}_{t}")
                    nc.vector.tensor_mul(msq[:], mv[:, 0:1], mv[:, 0:1])
                    nc.vector.tensor_add(mv[:, 1:2], mv[:, 1:2], msq[:])

            def gn_mid(b):
                """one block-diag selector matmul -> per-channel group stats
                (already broadcast), then [128, t] affine columns."""
                st_ = S[b]
                gs_ps = ppsp.tile([128, NC2, 2], F32, tag="p", name=f"gsps{b}")
                # sel is host-scaled by 1/32, so gs_ps holds the per-group
                # [mean, E[x^2]] directly and DVE reads it straight from PSUM
                # (no ACT drain hop on this serial chain)
                nc.tensor.matmul(gs_ps.rearrange("p t s -> p (t s)"),
                                 sel[:, :],
                                 st_["stats"].rearrange("p t s -> p (t s)"),
                                 start=True, stop=True)
                # one contiguous DVE drain (strided PSUM reads are rejected
                # by the BIR verifier), then tiny SBUF ops
                gsb = smalls.tile([128, NC2, 2], F32, tag="gsb", name=f"gsb{b}")
                nc.vector.tensor_copy(gsb.rearrange("p t s -> p (t s)"),
                                      gs_ps.rearrange("p t s -> p (t s)"))
                mm2 = smalls.tile([128, NC2], F32, tag="mm2", name=f"mm2{b}")
                nc.vector.tensor_mul(mm2[:], gsb[:, :, 0], gsb[:, :, 0])
                var = smalls.tile([128, NC2], F32, tag="var", name=f"var{b}")
                nc.vector.tensor_sub(var[:], gsb[:, :, 1], mm2[:])
                # rsqrt as exp(-0.5 ln(var+eps)): stays on the one loaded
                # Exp/Ln ACT table (Sqrt would force a table swap per sample)
                nc.scalar.activation(var[:], var[:], AF.Ln, bias=eps_t[:])
                a_cols = smalls.tile([128, NC2], F32, tag="acol", name=f"ac{b}")
                nc.scalar.activation(a_cols[:], var[:], AF.Exp, scale=-0.5)
                nc.vector.tensor_mul(a_cols[:], a_cols[:], gamma[:])
                c_cols = smalls.tile([128, NC2], F32, tag="ccol", name=f"cc{b}")
                nc.vector.tensor_mul(c_cols[:], gsb[:, :, 0], a_cols[:])
                nc.vector.tensor_sub(c_cols[:], beta[:], c_cols[:])
                st_["cols"] = (a_cols, c_cols)

            def gn_b(b):
                """h = a*x + c, written into a single [128, 2, N] fp8 tile
                (the layout DoubleRow matmuls contract K=256 over)."""
                st_ = S[b]
                a_cols, c_cols = st_["cols"]
                h2 = hp.tile([128, NC2, N], FP8, tag="h", name=f"h{b}")
                # half-width ops, h-half outer: the u projection's first
                # 512-chunk unblocks after only two of the four ops.  Sample
                # 0 puts the t=1 halves on the (initialized, idle) ACT.
                for hh in range(2):
                    hsl = slice(hh * 512, (hh + 1) * 512)
                    for t in range(NC2):
                        if b == 0 and t == 1:
                            nc.scalar.activation(h2[:, t, hsl],
                                                 st_["x"][t][:, hsl],
                                                 AF.Identity,
                                                 bias=c_cols[:, t:t + 1],
                                                 scale=a_cols[:, t:t + 1])
                        else:
                            nc.vector.tensor_scalar(h2[:, t, hsl],
                                                    st_["x"][t][:, hsl],
                                                    a_cols[:, t:t + 1],
                                                    c_cols[:, t:t + 1],
                                                    op0=OP.mult, op1=OP.add)
                st_["h2"] = h2

            def proj_pieces(b, ring, ring_tag, ring_v=None, tag_v=None):
                if ring_v is None:
                    ring_v, tag_v = ring, ring_tag
                """u (fused qk) or q,k, plus M = (wov h)^T, all fp8 — as a
                list of zero-arg emit callables so the pieces can be
                interleaved between att matmuls (each piece ping-pongs one
                1-bank PSUM slot against its DVE drain copy)."""
                st_ = S[b]
                pieces = []

                def qk_piece(wT, bias_cols, qktag, dst, mt, icc):
                    def go():
                        h2 = st_["h2"]   # set by gn_b, possibly a later piece
                        osl = slice(icc * 512, (icc + 1) * 512)
                        ps = ring.tile([128, 512], F32, tag=ring_tag,
                                       name=f"{qktag}ps{b}_{mt}_{icc}")
                        nc.tensor.matmul(
                            ps[:], wT[:, :, mt * 128:(mt + 1) * 128],
                            h2[:, :, osl], perf_mode=DR,
                            start=True, stop=True)
                        if bias_cols is None:
                            # tensor_copy hits the DVE copy perf mode
                            nc.vector.tensor_copy(dst[:, mt, osl], ps[:])
                        else:
                            nc.vector.tensor_scalar_add(
                                dst[:, mt, osl], ps[:], bias_cols[:, mt:mt + 1])
                    return go

                def qk(wT, bias_cols, qktag):
                    dst = qkp.tile([128, NC2, N], FP8, tag=qktag, name=f"{qktag}{b}")
                    # icc outer: the first att half (i < 512) only needs the
                    # icc=0 pieces of both mt chunks
                    for icc in range(NIC):
                        for mt in range(NC2):
                            pieces.append(qk_piece(wT, bias_cols, qktag,
                                                   dst, mt, icc))
                    return dst

                if fused_qk:
                    st_["mov"] = qk(wuT, None, "u")
                    st_["sta"] = None   # resolved to h2 in att_h
                else:
                    st_["mov"] = qk(wqT, bqv if with_qk_bias else None, "u")
                    st_["sta"] = qk(wkT, bkv if with_qk_bias else None, "k")
                # M in fp8 [128, jpair, 2, C]; two j's share one PSUM tile so
                # the drain is a single 512-wide DVE copy per pair
                vT = vtp.tile([128, NJ // 2, 2, C], FP8, tag="vt", name=f"vt{b}")

                def vt_piece(jp):
                    def go():
                        h2 = st_["h2"]
                        ps = ring_v.tile([128, 2, C], F32, tag=tag_v,
                                         name=f"vtps{b}_{jp}")
                        for s in range(2):
                            j = 2 * jp + s
                            nc.tensor.matmul(ps[:, s, :],
                                             h2[:, :, j * 128:(j + 1) * 128],
                                             wovT[:, :, :], perf_mode=DR,
                                             start=True, stop=True)
                        nc.vector.tensor_copy(
                            vT[:, jp, :, :].rearrange("p s c -> p (s c)"),
                            ps.rearrange("p s c -> p (s c)"))
                    return go

                for jp in range(NJ // 2):
                    pieces.append(vt_piece(jp))
                st_["vT"] = vT
                return pieces

            NP = NJ // 2

            def att_h(b, ic, pieces=(), schedule=(), pre_sp=None,
                      prefetch_next=False):
                """one i-half of attention: scores^T pairs -> single
                1024-wide exp -> output accumulation, then the broadcast row
                sums in one LdW-deduped ones-matmul chain.  `pieces` are
                gn/proj emit callables slotted after each jp's matmuls
                (schedule[jp] = how many); a piece at slot jp must only
                produce data consumed at jp+1 or later.  Leftovers are
                returned for the caller to place.  With prefetch_next, the
                NEXT half's first sT pair is emitted into the exp(jp3) wait
                window before the row-sum chain (it has no exp dependency)
                and handed back for that half to consume via pre_sp."""
                pieces = list(pieces)
                st_ = S[b]
                sta, mov, vT = st_["sta"], st_["mov"], st_["vT"]
                if sta is None:
                    sta = st_["h2"]
                isl = slice(ic * 512, (ic + 1) * 512)
                ou = accp.tile([128, NC2, 512], F32, tag="ou",
                               name=f"ou{b}_{ic}")
                r_ps = rpsp.tile([128, 512], F32, tag="r", name=f"rps{b}_{ic}")

                def emit_pair(jp, picl=ic):
                    isl_ = slice(picl * 512, (picl + 1) * 512)
                    sp = psp.tile([128, 2, 512], F32, tag="s",
                                  name=f"sT{b}_{picl}_{jp}")
                    for s2 in range(2):
                        j = 2 * jp + s2
                        nc.tensor.matmul(sp[:, s2, :],
                                         sta[:, :, j * 128:(j + 1) * 128],
                                         mov[:, :, isl_], perf_mode=DR,
                                         start=True, stop=True)
                    return sp

                tail = (b == NB - 1 and ic == 1)
                sched = dict(schedule)
                sp = pre_sp if pre_sp is not None else emit_pair(0)
                pu2s = []

                def emit_rsums():
                    # back-to-back so the LdW dedupe keeps one ones
                    # weight-load per half
                    for jp in range(NP):
                        nc.tensor.matmul(r_ps[:], ones128[:], pu2s[jp][:],
                                         perf_mode=DR,
                                         start=(jp == 0), stop=(jp == NP - 1))

                for jp in range(NP):
                    pu2 = pup.tile([128, 2, 512], FP8, tag="pu",
                                   name=f"pu{b}_{ic}_{jp}")
                    nc.scalar.activation(pu2.rearrange("p s i -> p (s i)"),
                                         sp.rearrange("p s i -> p (s i)"),
                                         AF.Exp, scale=scale_exp)
                    pu2s.append(pu2)
                    if jp + 1 < NP:
                        sp = emit_pair(jp + 1)
                    if tail and jp == NP - 1:
                        # final half: row sums first so epi's ln/exp overlap
                        # the remaining output matmuls
                        emit_rsums()
                    for ct in range(NC2):
                        nc.tensor.matmul(ou[:, ct, :],
                                         vT[:, jp, :, ct * 128:(ct + 1) * 128],
                                         pu2[:], perf_mode=DR,
                                         start=(jp == 0), stop=(jp == NP - 1))
                    for _ in range(sched.get(jp, 0)):
                        if pieces:
                            pieces.pop(0)()
                next_sp = emit_pair(0, ic + 1) if prefetch_next else None
                if not tail:
                    emit_rsums()
                st_[f"ou{ic}"] = ou
                st_[f"r{ic}"] = r_ps
                return pieces, next_sp

            def epi_h(b, ic):
                """1/(16 r) = exp(-ln r - ln 16) on the broadcast row sums,
                then normalize, residual, store for this i-half."""
                st_ = S[b]
                ou, r_ps, xt = st_[f"ou{ic}"], st_[f"r{ic}"], st_["x"]
                isl = slice(ic * 512, (ic + 1) * 512)
                lnr = rp.tile([128, 512], F32, tag="lnr", name=f"lnr{b}_{ic}")
                rinv = rp.tile([128, 512], F32, tag="rinv", name=f"rinv{b}_{ic}")
                nc.scalar.activation(lnr[:], r_ps[:], AF.Ln)
                # bias: wov is host-scaled by WSCALE -> fold 1/WSCALE here
                nc.scalar.activation(rinv[:], lnr[:], AF.Exp, scale=-1.0,
                                     bias=mln16[:])
                last = (b == NB - 1)
                for ct in range(NC2):
                    fin = finp.tile([128, 512], F32, tag="fin",
                                    name=f"fin{b}_{ic}_{ct}")
                    src = ou[:, ct, :]
                    if with_bias_rank1:
                        # ou += Bf[o] * r[i] as a fused DVE op on the
                        # broadcast row sums (scores bias rank-1 term)
                        srcb = finp.tile([128, 512], F32, tag="finb",
                                         name=f"finb{b}_{ic}_{ct}")
                        nc.vector.scalar_tensor_tensor(
                            srcb[:], r_ps[:], Bfc[:, ct:ct + 1], src,
                            op0=OP.mult, op1=OP.add)
                        src = srcb[:]
                    nc.vector.tensor_tensor(fin[:], src, rinv[:], op=OP.mult)
                    if last:
                        # tail latency: keep the residual add on DVE
                        nc.vector.tensor_add(fin[:], fin[:], xt[ct][:, isl])
                    else:
                        # gpsimd add overlaps mid-kernel
                        nc.gpsimd.tensor_add(fin[:], fin[:], xt[ct][:, isl])
                    nc.sync.dma_start(out=out_d[b, ct * 128:(ct + 1) * 128, isl],
                                      in_=fin[:])

            # ---- pipelined emission ----
            # ic-outer halves.  Sample b+1's work drips into sample b's
            # stream as interleaved pieces: bn_stats before epi_h(b,0) (DVE
            # idles during the first half), the rest of GN + the u
            # projection inside att(b,1)'s jp slots, and the vT pieces carry
            # over into att(b+1,0)'s early slots (vT[jp] is consumed one
            # slot later than it is produced).
            gn_a1(0)
            gn_a2(0)
            gn_mid(0)
            gn_b(0)
            # startup proj: interleave u pieces (2-slot "s" ring) with vT
            # pieces ("p" ring) so the two psum slots ping-pong in parallel
            p0 = proj_pieces(0, psp, "s", ppsp, "p")
            nu = len(p0) - NJ // 2
            for i in range(max(nu, NJ // 2)):
                if i < nu:
                    p0[i]()
                if i < NJ // 2:
                    p0[nu + i]()
            carry = []
            for b in range(NB):
                carry, sp1 = att_h(b, 0, carry, {0: 1, 1: 1, 2: 1, 3: 99},
                                   prefetch_next=True)
                if b + 1 < NB:
                    # bn_stats + aggregation run on the idle DVE while this
                    # half's matmul/exp stream is in flight
                    gn_a1(b + 1)
                    gn_a2(b + 1)
                epi_h(b, 0)
                if b + 1 < NB:
                    bn = b + 1

                    def gn_piece1(bn=bn):
                        gn_mid(bn)

                    def gn_piece2(bn=bn):
                        gn_b(bn)

                    pieces = [gn_piece1, gn_piece2]
                    pieces += proj_pieces(b + 1, ppsp, "p")
                else:
                    pieces = []
                carry, _ = att_h(b, 1, pieces, {0: 2, 1: 2, 2: 2, 3: 3},
                                 pre_sp=sp1)
                epi_h(b, 1)
            for pc in carry:
                pc()

    return nc


_cache = {}


def kernel(x, gamma, beta, wq, bq, wk, bk, wv, bv, wo, bo):
    """Full inputs -> full output. Shards batch 4/core over 8 cores."""
    _install()
    from concourse.bass_utils import run_bass_kernel_spmd

    x = np.asarray(x)
    B, Cc, H, W = x.shape
    assert (Cc, H * W) == (C, N) and B == NB * NCORES
    xf = np.ascontiguousarray(x.reshape(B, C, N).astype(np.float32))

    wq = np.asarray(wq); wk = np.asarray(wk); wv = np.asarray(wv); wo = np.asarray(wo)
    bq = np.asarray(bq); bk = np.asarray(bk); bv = np.asarray(bv); bo = np.asarray(bo)
    gamma = np.asarray(gamma); beta = np.asarray(beta)

    Bf = (wo.astype(np.float64) @ bv.astype(np.float64) + bo).astype(np.float32)
    wov = (wo.astype(np.float64) @ wv.astype(np.float64)) * WSCALE
    has_bias = bool(np.any(Bf != 0.0))
    has_qk_bias = bool(np.any(bq != 0.0) or np.any(bk != 0.0))
    fused_qk = not has_qk_bias

    # block-diag group selector pre-scaled by 1/32 (channels per group) so
    # the group-sum matmul yields means directly
    sel128 = np.zeros((128, 128), np.float32)
    for p in range(128):
        g0 = (p // 32) * 32
        sel128[p, g0:g0 + 32] = 1.0 / 32.0

    import ml_dtypes
    FP8NP = ml_dtypes.float8_e4m3

    def stage_w(wT):
        # [C(a), C(b)] -> [128, 2, C] fp8: a = (t p) split across tile dim
        return np.ascontiguousarray(
            wT.astype(np.float32).reshape(NC2, 128, C).transpose(1, 0, 2)
        ).astype(FP8NP)

    common = {
        "wovT": stage_w(wov.T),
        "gamma": gamma.astype(np.float32), "beta": beta.astype(np.float32),
        "sel128": sel128,
    }
    if fused_qk:
        wu = (wq.astype(np.float64).T @ wk.astype(np.float64)) * WSCALE
        common["wuT"] = stage_w(wu)
    else:
        common["wqT"] = stage_w(wq.T)
        common["wkT"] = stage_w(wk.T)
        common["bq"] = bq.astype(np.float32)
        common["bk"] = bk.astype(np.float32)
    if has_bias:
        # the rank-1 term rides through the 1/WSCALE folded into rinv
        common["Bf"] = Bf * WSCALE
    in_maps = []
    for c in range(NCORES):
        m = dict(common)
        m["xs"] = np.ascontiguousarray(xf[c * NB:(c + 1) * NB])
        in_maps.append(m)

    key = (has_bias, has_qk_bias)
    if key not in _cache:
        _cache[key] = build_kernel(with_bias_rank1=has_bias,
                                   with_qk_bias=has_qk_bias,
                                   fused_qk=fused_qk)
    nc = _cache[key]

    trace = os.environ.get("TRN_KERNEL_TRACE", "0") == "1"
    kw = {}
    if trace:
        import shutil, tempfile
        td = os.environ.get("TRN_KERNEL_TRACE_DIR") or tempfile.mkdtemp()
        shutil.rmtree(td, ignore_errors=True)
        os.makedirs(td, exist_ok=True)
        kw = dict(trace=True, tmpdir=td)
    res = run_bass_kernel_spmd(nc, in_maps, list(range(NCORES)), **kw)
    _last_exec_time_ns[0] = getattr(res, "exec_time_ns", None)

    full = np.concatenate([res.results[c]["out"] for c in range(NCORES)], axis=0)
    return full.reshape(B, C, H, W).astype(np.float32)


def last_exec_time_ns():
    return _last_exec_time_ns[0]
